# revision 1
# baseline (speedup 1.0000x reference)
"""Trainium2 Bass kernel for multi-head self-attention with RoPE.

Problem shapes (hardcoded): x [2, 2048, 1024], 16 heads x 64 dim, fp32.
Sharding: tensor-parallel over heads -- each of 8 cores owns 2 heads.
Each core computes q/k/v projections for its heads, RoPE, attention
(flash-style, transposed layout), and its partial output projection.
Host sums the 8 partial outputs and adds bo.

Device-side layout notes:
 - All matmul contractions need the contracted dim on SBUF partitions, so
   the host passes x pre-transposed (xT [B, D, T]).
 - Q^T/K^T are produced in [head_dim, T] layout; RoPE uses an interleaved
   pair layout (W columns permuted on host) so the rotation partner of
   partition p is partition p^1, reachable by stream_shuffle's within-32
   permutation.
 - Scores are computed transposed (S^T = K Q^T, [k, q]); softmax denominators
   come from an extra ones-column in the PV matmul; normalization is
   reciprocal + gpsimd partition_broadcast + multiply.
"""

import os
import numpy as np

import concourse.bass as bass
import concourse.tile as tile
from concourse import mybir
from concourse.bass_utils import run_bass_kernel_spmd

N_CORES = 8
B, T, D = 2, 2048, 1024
H, HD = 16, 64            # total heads, head dim
HL = H // N_CORES         # heads per core (2)
DL = HL * HD              # local head dims (128)
CC = D // 128             # contraction chunks (8)
NT = T // 512             # 512-wide t-chunks per batch (4)
NKT = T // 128            # 128-row k-tiles per batch (16)
F32 = mybir.dt.float32
FR = mybir.dt.float32r

# within-32 adjacent-pair swap for RoPE (partition p <-> p^1)
SWAP_MASK = [i ^ 1 for i in range(32)]

_CACHE = {}
LAST_RESULT = None


def _build_nc(dbg_names=()):
    from concourse import bacc
    nc = bacc.Bacc("TRN2", target_bir_lowering=False, debug=False,
                   num_devices=N_CORES)
    xt = nc.dram_tensor("xt", [B, D, T], FR, kind="ExternalInput").ap()
    wq = nc.dram_tensor("wq", [D, DL], FR, kind="ExternalInput").ap()
    wk = nc.dram_tensor("wk", [D, DL], FR, kind="ExternalInput").ap()
    wv = nc.dram_tensor("wv", [D, DL], FR, kind="ExternalInput").ap()
    # natural Wo row-slice for this core: [128, 1024]
    wo2 = nc.dram_tensor("wo2", [DL, D], FR, kind="ExternalInput").ap()
    cosb = nc.dram_tensor("cosb", [128, T], F32, kind="ExternalInput").ap()
    sinb = nc.dram_tensor("sinb", [128, T], F32, kind="ExternalInput").ap()
    # columns: bq, bq_shuf, bk, bk_shuf
    bqk = nc.dram_tensor("bqk", [128, 4], F32, kind="ExternalInput").ap()
    bv1 = nc.dram_tensor("bv1", [128, 1], F32, kind="ExternalInput").ap()
    out = nc.dram_tensor("out", [B * T, D], F32, kind="ExternalOutput").ap()

    dbg = {}
    dbg_shapes = {
        "dbg_qrot": [128, T], "dbg_krot": [128, T], "dbg_vh": [128, 1040],
        "dbg_sp": [128, 1024], "dbg_pt": [128, 1024], "dbg_cx": [65, 512],
        "dbg_rcp": [1, 512], "dbg_bc": [64, 512], "dbg_cn": [64, 512],
    }
    for n in dbg_names:
        dbg[n] = nc.dram_tensor(n, dbg_shapes[n], F32,
                                kind="ExternalOutput").ap()

    with tile.TileContext(nc) as tc:
        _body(tc, xt, wq, wk, wv, wo2, cosb, sinb, bqk, bv1, out, dbg)

    nc.compile()
    return nc


def _body(tc, xt, wq, wk, wv, wo2, cosb, sinb, bqk, bv1, out, dbg={}):
    nc = tc.nc
    from contextlib import ExitStack
    with ExitStack() as ctx:
        consts = ctx.enter_context(tc.tile_pool(name="consts", bufs=1))
        xt_pool = ctx.enter_context(tc.tile_pool(name="xt", bufs=2))
        qkv_pool = ctx.enter_context(tc.tile_pool(name="qkv", bufs=2))
        p_pool = ctx.enter_context(tc.tile_pool(name="pp", bufs=4))
        small_pool = ctx.enter_context(tc.tile_pool(name="sm", bufs=3))
        norm_pool = ctx.enter_context(tc.tile_pool(name="nrm", bufs=2))
        ctxn_pool = ctx.enter_context(tc.tile_pool(name="ctxn", bufs=4))
        out_pool = ctx.enter_context(tc.tile_pool(name="outp", bufs=3))
        ps_small = ctx.enter_context(
            tc.tile_pool(name="pss", bufs=2, space="PSUM"))
        ps_s = ctx.enter_context(tc.tile_pool(name="pse", bufs=2, space="PSUM"))
        ps_ctx = ctx.enter_context(
            tc.tile_pool(name="psc", bufs=2, space="PSUM"))

        # ---- constants ----
        wq_sb = consts.tile([128, CC * DL], FR)
        nc.sync.dma_start(
            wq_sb[:, :].rearrange("p (cc m) -> p cc m", cc=CC),
            wq.rearrange("(cc p) m -> p cc m", p=128))
        wk_sb = consts.tile([128, CC * DL], FR)
        nc.sync.dma_start(
            wk_sb[:, :].rearrange("p (cc m) -> p cc m", cc=CC),
            wk.rearrange("(cc p) m -> p cc m", p=128))
        wv_sb = consts.tile([128, CC * DL], FR)
        nc.sync.dma_start(
            wv_sb[:, :].rearrange("p (cc m) -> p cc m", cc=CC),
            wv.rearrange("(cc p) m -> p cc m", p=128))
        wo2_sb = consts.tile([DL, D], FR)
        nc.sync.dma_start(wo2_sb[:, :], wo2)
        cos_sb = consts.tile([128, T], F32)
        nc.sync.dma_start(cos_sb[:, :], cosb)
        sin_sb = consts.tile([128, T], F32)
        nc.sync.dma_start(sin_sb[:, :], sinb)
        bqk_sb = consts.tile([128, 4], F32)
        nc.sync.dma_start(bqk_sb[:, :], bqk)
        bv_sb = consts.tile([128, 1], F32)
        nc.sync.dma_start(bv_sb[:, :], bv1)
        ident = consts.tile([128, 128], F32)
        from concourse.masks import make_identity
        make_identity(nc, ident[:, :])
        ones16 = consts.tile([128, NKT], F32)
        nc.gpsimd.memset(ones16[:, :], 1.0)

        for b in range(B):
            # ======== projection + rope + V transpose phase ========
            qrot = qkv_pool.tile([128, T], FR, tag="qrot")
            krot = qkv_pool.tile([128, T], FR, tag="krot")
            vh = [qkv_pool.tile([128, NKT * (HD + 1)], FR, tag=f"vh{h}",
                                name=f"vh{h}_{b}")
                  for h in range(HL)]
            for h in range(HL):
                # ones column (col 64 of every 65-wide block) for softmax sums
                nc.vector.tensor_copy(
                    vh[h][:, :].rearrange("p (kt c) -> p kt c", c=HD + 1)
                    [:, :, HD:HD + 1],
                    ones16[:, :].rearrange("p (kt o) -> p kt o", o=1))

            for tcn in range(NT):
                ts = slice(tcn * 512, (tcn + 1) * 512)
                xt_sb = xt_pool.tile([128, CC * 512], FR, tag="xt")
                xt_src = xt[b, :, ts].rearrange("(cc p) t -> p cc t", p=128)
                for ci in range(CC):
                    nc.sync.dma_start(
                        xt_sb[:, ci * 512:(ci + 1) * 512], xt_src[:, ci, :])

                for name, w_sb in (("q", wq_sb), ("k", wk_sb), ("v", wv_sb)):
                    pp = ps_small.tile([128, 512], F32, tag="pss",
                                       name=f"pp_{name}_{b}_{tcn}")
                    for ci in range(CC):
                        nc.tensor.matmul(
                            pp[:, :],
                            w_sb[:, ci * DL:(ci + 1) * DL],
                            xt_sb[:, ci * 512:(ci + 1) * 512],
                            start=(ci == 0), stop=(ci == CC - 1))
                    if name == "v":
                        # V^T: add bias, park in SBUF for PE transposes
                        vt_sb = small_pool.tile([128, 512], F32, tag="vt")
                        nc.vector.tensor_scalar_add(vt_sb[:, :], pp[:, :],
                                                    bv_sb[:, 0:1])
                        for j in range(4):
                            kt = tcn * 4 + j
                            tp = ps_small.tile([128, 128], F32, tag="pss",
                                               name=f"tp_{b}_{kt}")
                            nc.tensor.transpose(
                                tp[:, :], vt_sb[:, j * 128:(j + 1) * 128],
                                ident[:, :])
                            for h in range(HL):
                                nc.vector.tensor_copy(
                                    vh[h][:, kt * 65: kt * 65 + 64],
                                    tp[:, h * HD:(h + 1) * HD])
                    else:
                        # rope: rot = (x + b) * cos + (swap(x) + swap(b)) * sin
                        dst = qrot if name == "q" else krot
                        bcol = 0 if name == "q" else 2
                        shuf = small_pool.tile([128, 512], F32, tag="shuf")
                        nc.vector.stream_shuffle(shuf[:, :], pp[:, :],
                                                 SWAP_MASK)
                        ca = small_pool.tile([128, 512], F32, tag="ca")
                        nc.vector.scalar_tensor_tensor(
                            ca[:, :], pp[:, :], bqk_sb[:, bcol:bcol + 1],
                            cos_sb[:, ts],
                            op0=mybir.AluOpType.add,
                            op1=mybir.AluOpType.mult)
                        sa = small_pool.tile([128, 512], F32, tag="sa")
                        nc.vector.scalar_tensor_tensor(
                            sa[:, :], shuf[:, :],
                            bqk_sb[:, bcol + 1:bcol + 2],
                            sin_sb[:, ts],
                            op0=mybir.AluOpType.add,
                            op1=mybir.AluOpType.mult)
                        nc.vector.tensor_add(dst[:, ts], ca[:, :], sa[:, :])

            if b == 0:
                if "dbg_qrot" in dbg:
                    nc.sync.dma_start(dbg["dbg_qrot"], qrot[:, :].bitcast(F32))
                if "dbg_krot" in dbg:
                    nc.sync.dma_start(dbg["dbg_krot"], krot[:, :].bitcast(F32))
                if "dbg_vh" in dbg:
                    nc.sync.dma_start(dbg["dbg_vh"], vh[0][:, :].bitcast(F32))

            # ======== attention phase (flash-style over 512-wide q chunks)
            # h0/h1 S matmuls sit in row groups 0-63 / 64-127 and overlap on
            # the PE; one 1024-wide exp covers both heads per k-tile.
            for qc in range(4):
                cx = [ps_ctx.tile([HD + 1, 512], F32, tag="ctx",
                                  name=f"cx_{b}_{qc}_{h}") for h in range(HL)]
                qs = slice(qc * 512, (qc + 1) * 512)
                for kt in range(NKT):
                    sp = ps_s.tile([128, 1024], F32, tag="s",
                                   name=f"sp_{b}_{qc}_{kt}")
                    for h in range(HL):
                        hs = slice(h * HD, (h + 1) * HD)
                        nc.tensor.matmul(
                            sp[:, h * 512:(h + 1) * 512],
                            krot[hs, kt * 128:(kt + 1) * 128],
                            qrot[hs, qs], start=True, stop=True)
                    pt = p_pool.tile([128, 1024], FR, tag="pt")
                    nc.scalar.activation(
                        pt[:, :], sp[:, :],
                        mybir.ActivationFunctionType.Exp,
                        scale=1.0 / np.sqrt(HD).item())
                    if b == 0 and qc == 0 and kt == 0 and "dbg_pt" in dbg:
                        tpt = small_pool.tile([128, 1024], F32, tag="tpt",
                                              bufs=1)
                        nc.vector.tensor_copy(tpt[:, :], pt[:, :])
                        nc.sync.dma_start(dbg["dbg_pt"], tpt[:, :])
                    for h in range(HL):
                        nc.tensor.matmul(
                            cx[h][:, :],
                            vh[h][:, kt * 65:(kt + 1) * 65],
                            pt[:, h * 512:(h + 1) * 512],
                            start=(kt == 0), stop=(kt == NKT - 1))

                # softmax normalize; stack both heads' ctx^T into [128, 512]
                stk = ctxn_pool.tile([128, 512], FR, tag="stk")
                for h in range(HL):
                    den = norm_pool.tile([HD + 1, 512], F32, tag="den")
                    nc.scalar.copy(den[HD:HD + 1, :], cx[h][HD:HD + 1, :])
                    den0 = norm_pool.tile([1, 512], F32, tag="den0")
                    nc.sync.dma_start(den0[0:1, :], den[HD:HD + 1, :])
                    rcp = norm_pool.tile([1, 512], F32, tag="rcp")
                    nc.vector.reciprocal_approx_fast(rcp[0:1, :], den0[0:1, :])
                    bc = norm_pool.tile([HD, 512], F32, tag="bc")
                    nc.gpsimd.partition_broadcast(bc[:, :], rcp[0:1, :],
                                                  channels=HD)
                    if h == 0:
                        nc.vector.tensor_mul(stk[0:HD, :], cx[h][0:HD, :],
                                             bc[:, :])
                    else:
                        cn1 = norm_pool.tile([HD, 512], FR, tag="cn1")
                        nc.vector.tensor_mul(cn1[:, :], cx[h][0:HD, :],
                                             bc[:, :])
                        nc.sync.dma_start(stk[HD:128, :], cn1[:, :])
                    if b == 0 and qc == 0 and h == 0:
                        if "dbg_cx" in dbg:
                            tcx = small_pool.tile([HD + 1, 512], F32,
                                                  tag="tcx", bufs=1)
                            nc.vector.tensor_copy(tcx[:, :], cx[h][:, :])
                            nc.sync.dma_start(dbg["dbg_cx"], tcx[:, :])
                        if "dbg_rcp" in dbg:
                            nc.sync.dma_start(dbg["dbg_rcp"], rcp[0:1, :])
                        if "dbg_bc" in dbg:
                            nc.sync.dma_start(dbg["dbg_bc"], bc[:, :])
                        if "dbg_cn" in dbg:
                            nc.sync.dma_start(dbg["dbg_cn"],
                                              stk[0:HD, :].bitcast(F32))

                # ---- output projection for this q chunk (K=128) ----
                for tsub in range(4):
                    row0 = b * T + qc * 512 + tsub * 128
                    osb = out_pool.tile([128, D], F32, tag="osb")
                    for dc in range(2):
                        op = ps_small.tile([128, 512], F32, tag="pss",
                                           name=f"op_{row0}_{dc}")
                        nc.tensor.matmul(
                            op[:, :],
                            stk[:, tsub * 128:(tsub + 1) * 128],
                            wo2_sb[:, dc * 512:(dc + 1) * 512],
                            start=True, stop=True)
                        nc.any.tensor_copy(osb[:, dc * 512:(dc + 1) * 512],
                                           op[:, :])
                    nc.sync.dma_start(out[row0:row0 + 128, :], osb[:, :])


def _rope_tables():
    """cos/sin tables in the interleaved-pair partition layout, fp32 math
    to match the reference."""
    pos = np.arange(T, dtype=np.float32)[:, None]                 # [T, 1]
    freq_seq = np.arange(HD // 2, dtype=np.float32)
    inv_freq = (1.0 / (10000.0 ** (freq_seq / np.float32(HD // 2)))).astype(
        np.float32)
    ang = pos * inv_freq[None, :]                                 # [T, 32]
    sin = np.sin(ang).astype(np.float32)                          # [T, 32]
    cos = np.cos(ang).astype(np.float32)
    cosb = np.empty((128, T), dtype=np.float32)
    sinb = np.empty((128, T), dtype=np.float32)
    for p in range(128):
        r = p % HD
        j = r // 2
        second = r % 2
        cosb[p] = cos[:, j]
        sinb[p] = sin[:, j] if second else -sin[:, j]
    return cosb, sinb


def _perm():
    """interleaved-pair permutation of each head's 64 dims:
    new[h*64 + 2j] = old[h*64 + j]; new[h*64 + 2j + 1] = old[h*64 + 32 + j]"""
    p = np.arange(DL)
    return (p // HD) * HD + (p % HD) // 2 + (p % 2) * (HD // 2)


def kernel(**inputs):
    global LAST_RESULT
    x = np.ascontiguousarray(np.asarray(inputs["x"], dtype=np.float32))
    Wq = np.asarray(inputs["Wq"], dtype=np.float32)
    Wk = np.asarray(inputs["Wk"], dtype=np.float32)
    Wv = np.asarray(inputs["Wv"], dtype=np.float32)
    Wo = np.asarray(inputs["Wo"], dtype=np.float32)
    bq = np.asarray(inputs["bq"], dtype=np.float32)
    bk = np.asarray(inputs["bk"], dtype=np.float32)
    bv = np.asarray(inputs["bv"], dtype=np.float32)
    bo = np.asarray(inputs["bo"], dtype=np.float32)

    if "nc" not in _CACHE:
        _CACHE["nc"] = _build_nc()
    nc = _CACHE["nc"]

    xT = np.ascontiguousarray(x.transpose(0, 2, 1))               # [B, D, T]
    cosb, sinb = _rope_tables()
    perm = _perm()
    swap = np.arange(128) ^ 1

    in_maps = []
    for c in range(N_CORES):
        cs = slice(c * DL, (c + 1) * DL)
        wq_c = np.ascontiguousarray(Wq[:, cs][:, perm])
        wk_c = np.ascontiguousarray(Wk[:, cs][:, perm])
        wv_c = np.ascontiguousarray(Wv[:, cs])
        wo_c = Wo[cs, :]
        bq_c = bq[cs][perm]
        bk_c = bk[cs][perm]
        bqk_c = np.stack([bq_c, bq_c[swap], bk_c, bk_c[swap]],
                         axis=1).astype(np.float32)
        in_maps.append({
            "xt": xT,
            "wq": wq_c, "wk": wk_c, "wv": wv_c,
            "wo2": np.ascontiguousarray(wo_c),
            "cosb": cosb, "sinb": sinb,
            "bqk": np.ascontiguousarray(bqk_c),
            "bv1": np.ascontiguousarray(bv[cs].reshape(128, 1)),
        })

    trace = bool(int(os.environ.get("BASS_KERNEL_TRACE", "0")))
    res = run_bass_kernel_spmd(nc, in_maps, core_ids=list(range(N_CORES)),
                               trace=trace)
    LAST_RESULT = res

    acc = np.zeros((B * T, D), dtype=np.float64)
    for r in res.results:
        acc += r["out"].astype(np.float64)
    out = (acc + bo.astype(np.float64)).astype(np.float32)
    return out.reshape(B, T, D)

